# revision 6
# baseline (speedup 1.0000x reference)
"""Trainium2 Bass kernel for nn_Conv2d_Local (locally-connected conv, untied
weights).

Problem: x [B=128, 1, 560, 560]; weight [P*NF, 1, 28, 28] with P=39*39=1521
patch locations (stride 14, kernel 28), NF=64 filters; bias [P*NF, 1].
out[b, f*P+p] = sum_{kh,kw} x[b, i*14+kh, j*14+kw] * w[f*P+p, kh, kw] + bias.

Strategy: shard the 39 patch rows across 8 cores (5 rows each, row 39 padded).
Per patch p this is a GEMM patch[b, 784] @ w_p[784, 64]. The contraction is
chunked as 4 kh-groups of 7 rows x 2 kw-blocks of 14 cols (K=98 per chunk,
aligned to the stride so x chunks are shared between horizontally adjacent
patches). Adjacent patches' chunks that share the same x tile are paired into
one matmul of N=128 (two 64-wide weight halves -> two adjacent 64-col psum
slices), so each x tile is loaded stationary exactly once per patch row.

All device data is bf16 (inputs quantized host-side; psum accumulates fp32;
output stored bf16 and upcast on host), which quarters tensor-engine time and
halves HBM traffic vs fp32. Input DMAs ride both HWDGE rings: x slabs on the
SP ring (nc.sync), weights on the ACT ring (nc.scalar); output on SWDGE
(gpsimd). Weight DMAs are one 784KB transfer per (row, psum-bank); the first
row stays at pair granularity (and the first x slab is loaded in five column
slices) so compute starts early. Outputs drain per psum-bank as soon as each
bank's accumulation closes. Host pre-permutes x and w into DMA-friendly
layouts, adds the bias, and reassembles the final output in fp32.
"""
import sys

if '/opt/trn_rl_repo' not in sys.path:
    sys.path.insert(0, '/opt/trn_rl_repo')

import numpy as np

B = 128
H = W = 560
KH = KW = 28
DH = DW = 14
NF = 64
OH = OW = 39
P = OH * OW
NCORES = 8
NROWS = 5          # patch rows per core (40 total, row 39 is padding)
NGROUPS = 12       # 7-row kh-groups per core: rows 2*ri .. 2*ri+3 per patch row
GROWS = 82         # global 7-row groups covering 574 (padded) x rows
XSLAB_BUFS = 9
WH_BUFS = 20       # weight pair tiles [98, 2, 1024] bf16, 10 consumed per row
OROW_BUFS = 2
PSUM_BUFS = 8

_CACHE = {}


def build_program(repeats: int = 1):
    import concourse.bacc as bacc
    import concourse.mybir as mybir
    from concourse.tile import TileContext

    f32 = mybir.dt.float32
    bf16 = mybir.dt.bfloat16
    nc = bacc.Bacc("TRN2", target_bir_lowering=False, debug=False,
                   num_devices=NCORES)
    x_in = nc.dram_tensor("x", [NGROUPS, 98, 40, 128], bf16, kind="ExternalInput")
    w_in = nc.dram_tensor("w", [NROWS, 5, 98, 4, 1024], bf16, kind="ExternalInput")
    y_out = nc.dram_tensor("y", [NROWS, 128, OW * NF], bf16, kind="ExternalOutput")

    with TileContext(nc) as tc:
        with tc.tile_pool(name="xslab", bufs=XSLAB_BUFS) as xpool, \
             tc.tile_pool(name="wh", bufs=WH_BUFS) as wpool, \
             tc.tile_pool(name="op", bufs=OROW_BUFS) as opool, \
             tc.tile_pool(name="ps", bufs=PSUM_BUFS, space="PSUM") as pspool:
            def block_slices(mrel, npat):
                """(weight col slice, psum col slice) for block mrel of a bank."""
                if mrel == 0:
                    return (0, 64), (0, 64)
                if mrel < npat:
                    return ((128 * mrel - 64, 128 * mrel + 64),
                            (64 * (mrel - 1), 64 * (mrel + 1)))
                return ((128 * npat - 64, 128 * npat),
                        (64 * (npat - 1), 64 * npat))

            for _rep in range(repeats):
                xslabs = {}

                def load_slab(gi):
                    # x slabs ride the SP HWDGE ring (nc.sync); emit in
                    # consumption order so the per-ring FIFO matches demand.
                    # The very first slab lands in five column slices so the
                    # first matmul only gates on ~230KB.
                    if gi not in xslabs:
                        t = xpool.tile([98, 40, 128], bf16, tag="xslab",
                                       name=f"xs{gi}")
                        if gi == 0 and _rep == 0:
                            for lo, hi in ((0, 9), (9, 17), (17, 25),
                                           (25, 33), (33, 40)):
                                nc.sync.dma_start(out=t[:, lo:hi, :],
                                                  in_=x_in[gi, :, lo:hi, :])
                        else:
                            nc.sync.dma_start(out=t, in_=x_in[gi])
                        xslabs[gi] = t
                    return xslabs[gi]

                for ri in range(NROWS):
                    for g in range(2 * ri, min(2 * ri + 6, 2 * NROWS + 2)):
                        load_slab(g)
                    orow = opool.tile([128, OW * NF], bf16, tag="orow",
                                      name=f"orow{ri}")
                    psrow = [pspool.tile([128, 512], f32, tag="ps",
                                         name=f"ps{ri}_{t5}")
                             for t5 in range(5)]
                    wtiles = {}
                    for g4 in range(4):
                        pair, gg = divmod(g4, 2)
                        slab = xslabs[2 * ri + g4]
                        for t5 in range(5):
                            p0 = 8 * t5
                            npat = 8 if t5 < 4 else 7
                            npc = npat * 128
                            if gg == 0:
                                wt = wpool.tile([98, 2, 1024], bf16, tag="wh",
                                                name=f"wh{ri}_{t5}_{pair}")
                                # byte-balance the two HWDGE rings: the SP
                                # ring (x slabs, 12MB) also carries t5=4's
                                # weight tiles so both rings finish together
                                weng = nc.sync if t5 == 4 else nc.scalar
                                weng.dma_start(
                                    out=wt[:, :, :npc],
                                    in_=w_in[ri, t5, :, 2 * pair: 2 * pair + 2, :npc])
                                wtiles[(t5, pair)] = wt
                            wt = wtiles[(t5, pair)]
                            for mrel in range(npat + 1):
                                m = p0 + mrel
                                wsl, osl = block_slices(mrel, npat)
                                start = (g4 == 0 and mrel == 0)
                                stop = (g4 == 3 and mrel == npat)
                                nc.tensor.matmul(
                                    psrow[t5][:, osl[0]:osl[1]],
                                    slab[:, m, :],
                                    wt[:, gg, wsl[0]:wsl[1]],
                                    start=start, stop=stop)
                            if g4 == 3:
                                # drain this bank now: the cast overlaps the
                                # remaining banks' matmuls
                                nc.vector.tensor_copy(
                                    out=orow[:, 512 * t5: 512 * t5 + npat * 64],
                                    in_=psrow[t5][:, :npat * 64])
                    nc.gpsimd.dma_start(out=y_out[ri], in_=orow)
    nc.finalize()
    return nc


def _preprocess(x, weight):
    """Build per-core bf16 input maps from full x [B,1,560,560],
    weight [P*NF,1,28,28]."""
    import ml_dtypes
    bf16 = ml_dtypes.bfloat16

    x = np.asarray(x, dtype=np.float32).astype(bf16)
    weight = np.asarray(weight, dtype=np.float32).astype(bf16)

    # x -> pixel-major [574(pad), 560, 128], then 7-row slabs with partition
    # order (kh', kw'): [82, 98, 40, 128]
    xt = np.zeros((GROWS * 7, W, B), dtype=bf16)
    xt[:H] = x[:, 0].transpose(1, 2, 0)
    x_dev = np.ascontiguousarray(
        xt.reshape(GROWS, 7, 40, 14, B).transpose(0, 1, 3, 2, 4)
    ).reshape(GROWS, 98, 40, 128)

    # weight rows are f*P + p; reshape kh=(g,kh'), kw=(delta,kw') and order as
    # [i, k=(kh',kw'), g, cols=(j, delta, f)]
    w6 = weight.reshape(NF, OH, OW, 4, 7, 2, 14)
    w_flat = np.ascontiguousarray(
        w6.transpose(1, 4, 6, 3, 2, 5, 0)  # [i, kh', kw', g, j, delta, f]
    ).reshape(OH, 98, 4, OW * 2 * NF)

    w_dev = np.zeros((NROWS * NCORES, 5, 98, 4, 1024), dtype=bf16)
    for t5 in range(5):
        p0 = 8 * t5
        npat = 8 if t5 < 4 else 7
        w_dev[:OH, t5, :, :, :npat * 128] = \
            w_flat[:, :, :, 128 * p0: 128 * (p0 + npat)]

    in_maps = []
    for c in range(NCORES):
        in_maps.append({
            "x": np.ascontiguousarray(x_dev[10 * c: 10 * c + NGROUPS]),
            "w": np.ascontiguousarray(w_dev[NROWS * c: NROWS * (c + 1)]),
        })
    return in_maps


def _postprocess(results, bias):
    """results: list of per-core dicts with 'y' [NROWS, 128, OW*NF] bf16."""
    y = np.stack([np.asarray(r["y"], dtype=np.float32) for r in results])
    y = y.reshape(NCORES * NROWS, B, OW, NF)[:OH]    # [39, 128, 39, 64]
    out = np.ascontiguousarray(y.transpose(1, 3, 0, 2)).reshape(B, NF * P)
    out = out + np.asarray(bias, dtype=np.float32).reshape(1, NF * P)
    return out.reshape(B, NF * P, 1)


def kernel(x, weight, bias):
    from concourse.bass_utils import run_bass_kernel_spmd

    if "nc" not in _CACHE:
        _CACHE["nc"] = build_program()
    nc = _CACHE["nc"]
    in_maps = _preprocess(x, weight)
    res = run_bass_kernel_spmd(nc, in_maps, core_ids=list(range(NCORES)))
    return _postprocess(res.results, bias)


# revision 7
# speedup vs baseline: 1.0120x; 1.0120x over previous
"""Trainium2 Bass kernel for nn_Conv2d_Local (locally-connected conv, untied
weights).

Problem: x [B=128, 1, 560, 560]; weight [P*NF, 1, 28, 28] with P=39*39=1521
patch locations (stride 14, kernel 28), NF=64 filters; bias [P*NF, 1].
out[b, f*P+p] = sum_{kh,kw} x[b, i*14+kh, j*14+kw] * w[f*P+p, kh, kw] + bias.

Strategy: shard the 39 patch rows across 8 cores (5 rows each, row 39 padded).
Per patch p this is a GEMM patch[b, 784] @ w_p[784, 64]. The contraction is
chunked as 4 kh-groups of 7 rows x 2 kw-blocks of 14 cols (K=98 per chunk,
aligned to the stride so x chunks are shared between horizontally adjacent
patches). Adjacent patches' chunks that share the same x tile are paired into
one matmul of N=128 (two 64-wide weight halves -> two adjacent 64-col psum
slices), so each x tile is loaded stationary exactly once per patch row.

All device data is bf16 (inputs quantized host-side; psum accumulates fp32;
output stored bf16 and upcast on host), which quarters tensor-engine time and
halves HBM traffic vs fp32. Input DMAs ride both HWDGE rings: x slabs on the
SP ring (nc.sync), weights on the ACT ring (nc.scalar); output on SWDGE
(gpsimd). Weight DMAs are one 784KB transfer per (row, psum-bank); the first
row stays at pair granularity (and the first x slab is loaded in five column
slices) so compute starts early. Outputs drain per psum-bank as soon as each
bank's accumulation closes. Host pre-permutes x and w into DMA-friendly
layouts, adds the bias, and reassembles the final output in fp32.
"""
import sys

if '/opt/trn_rl_repo' not in sys.path:
    sys.path.insert(0, '/opt/trn_rl_repo')

import numpy as np

B = 128
H = W = 560
KH = KW = 28
DH = DW = 14
NF = 64
OH = OW = 39
P = OH * OW
NCORES = 8
NROWS = 5          # patch rows per core (40 total, row 39 is padding)
NGROUPS = 12       # 7-row kh-groups per core: rows 2*ri .. 2*ri+3 per patch row
GROWS = 82         # global 7-row groups covering 574 (padded) x rows
XSLAB_BUFS = 9
WH_BUFS = 20       # weight pair tiles [98, 2, 1024] bf16, 10 consumed per row
OROW_BUFS = 2
PSUM_BUFS = 8

_CACHE = {}


def build_program(repeats: int = 1):
    import concourse.bacc as bacc
    import concourse.mybir as mybir
    from concourse.tile import TileContext

    f32 = mybir.dt.float32
    bf16 = mybir.dt.bfloat16
    nc = bacc.Bacc("TRN2", target_bir_lowering=False, debug=False,
                   num_devices=NCORES)
    x_in = nc.dram_tensor("x", [NGROUPS, 98, 40, 128], bf16, kind="ExternalInput")
    w_in = nc.dram_tensor("w", [NROWS, 5, 98, 4, 1024], bf16, kind="ExternalInput")
    y_out = nc.dram_tensor("y", [NROWS, 128, OW * NF], bf16, kind="ExternalOutput")

    with TileContext(nc) as tc:
        with tc.tile_pool(name="xslab", bufs=XSLAB_BUFS) as xpool, \
             tc.tile_pool(name="wh", bufs=WH_BUFS) as wpool, \
             tc.tile_pool(name="op", bufs=OROW_BUFS) as opool, \
             tc.tile_pool(name="ps", bufs=PSUM_BUFS, space="PSUM") as pspool:
            def block_slices(mrel, npat):
                """(weight col slice, psum col slice) for block mrel of a bank."""
                if mrel == 0:
                    return (0, 64), (0, 64)
                if mrel < npat:
                    return ((128 * mrel - 64, 128 * mrel + 64),
                            (64 * (mrel - 1), 64 * (mrel + 1)))
                return ((128 * npat - 64, 128 * npat),
                        (64 * (npat - 1), 64 * npat))

            for _rep in range(repeats):
                xslabs = {}
                wtiles = {}

                def load_slab(gi, sliced=False):
                    # x slabs ride the SP HWDGE ring (nc.sync); emission is
                    # demand-ordered so the per-ring FIFO matches first use.
                    # The very first slab lands in five column slices so the
                    # first matmul only gates on ~230KB.
                    if gi in xslabs or gi > 2 * (NROWS - 1) + 3:
                        return
                    t = xpool.tile([98, 40, 128], bf16, tag="xslab",
                                   name=f"xs{gi}")
                    if sliced:
                        for lo, hi in ((0, 9), (9, 17), (17, 25),
                                       (25, 33), (33, 40)):
                            nc.sync.dma_start(out=t[:, lo:hi, :],
                                              in_=x_in[gi, :, lo:hi, :])
                    else:
                        nc.sync.dma_start(out=t, in_=x_in[gi])
                    xslabs[gi] = t

                def load_w(ri, t5, pair, eng):
                    # t5=4 weight tiles ride the SP ring to byte-balance the
                    # two HWDGE rings (ACT: 8 tiles/row, SP: 2 tiles + slabs)
                    npat = 8 if t5 < 4 else 7
                    npc = npat * 128
                    wt = wpool.tile([98, 2, 1024], bf16, tag="wh",
                                    name=f"wh{ri}_{t5}_{pair}")
                    eng.dma_start(
                        out=wt[:, :, :npc],
                        in_=w_in[ri, t5, :, 2 * pair: 2 * pair + 2, :npc])
                    wtiles[(ri, t5, pair)] = wt

                # row-0 prologue on the SP ring, in demand order
                load_slab(0, sliced=(_rep == 0))
                load_w(0, 4, 0, nc.sync)
                load_slab(1)
                load_w(0, 4, 1, nc.sync)
                for g in (2, 3, 4, 5):
                    load_slab(g)

                for ri in range(NROWS):
                    if ri >= 1:
                        # row segment on the SP ring: this row's t5=4 weights
                        # first (needed soonest), then next-row slab prefetch
                        load_w(ri, 4, 0, nc.sync)
                        load_w(ri, 4, 1, nc.sync)
                        load_slab(2 * ri + 4)
                        load_slab(2 * ri + 5)
                    last_row = (ri == NROWS - 1)
                    if not last_row:
                        orow = opool.tile([128, OW * NF], bf16, tag="orow",
                                          name=f"orow{ri}")
                    psrow = [pspool.tile([128, 512], f32, tag="ps",
                                         name=f"ps{ri}_{t5}")
                             for t5 in range(5)]
                    for g4 in range(4):
                        pair, gg = divmod(g4, 2)
                        slab = xslabs[2 * ri + g4]
                        for t5 in range(5):
                            p0 = 8 * t5
                            npat = 8 if t5 < 4 else 7
                            if gg == 0 and t5 < 4:
                                load_w(ri, t5, pair, nc.scalar)
                            wt = wtiles[(ri, t5, pair)]
                            for mrel in range(npat + 1):
                                m = p0 + mrel
                                wsl, osl = block_slices(mrel, npat)
                                start = (g4 == 0 and mrel == 0)
                                stop = (g4 == 3 and mrel == npat)
                                nc.tensor.matmul(
                                    psrow[t5][:, osl[0]:osl[1]],
                                    slab[:, m, :],
                                    wt[:, gg, wsl[0]:wsl[1]],
                                    start=start, stop=stop)
                            if g4 == 3:
                                # drain this bank now: the cast overlaps the
                                # remaining banks' matmuls; the last row also
                                # stores per-bank so the final DMA is small
                                if last_row:
                                    ot = opool.tile([128, 512], bf16,
                                                    tag="orow",
                                                    name=f"o{ri}_{t5}")
                                    nc.vector.tensor_copy(
                                        out=ot[:, :npat * 64],
                                        in_=psrow[t5][:, :npat * 64])
                                    nc.gpsimd.dma_start(
                                        out=y_out[ri, :,
                                                  512 * t5: 512 * t5 + npat * 64],
                                        in_=ot[:, :npat * 64])
                                else:
                                    nc.vector.tensor_copy(
                                        out=orow[:, 512 * t5: 512 * t5 + npat * 64],
                                        in_=psrow[t5][:, :npat * 64])
                    if not last_row:
                        nc.gpsimd.dma_start(out=y_out[ri], in_=orow)
    nc.finalize()
    return nc


def _preprocess(x, weight):
    """Build per-core bf16 input maps from full x [B,1,560,560],
    weight [P*NF,1,28,28]."""
    import ml_dtypes
    bf16 = ml_dtypes.bfloat16

    x = np.asarray(x, dtype=np.float32).astype(bf16)
    weight = np.asarray(weight, dtype=np.float32).astype(bf16)

    # x -> pixel-major [574(pad), 560, 128], then 7-row slabs with partition
    # order (kh', kw'): [82, 98, 40, 128]
    xt = np.zeros((GROWS * 7, W, B), dtype=bf16)
    xt[:H] = x[:, 0].transpose(1, 2, 0)
    x_dev = np.ascontiguousarray(
        xt.reshape(GROWS, 7, 40, 14, B).transpose(0, 1, 3, 2, 4)
    ).reshape(GROWS, 98, 40, 128)

    # weight rows are f*P + p; reshape kh=(g,kh'), kw=(delta,kw') and order as
    # [i, k=(kh',kw'), g, cols=(j, delta, f)]
    w6 = weight.reshape(NF, OH, OW, 4, 7, 2, 14)
    w_flat = np.ascontiguousarray(
        w6.transpose(1, 4, 6, 3, 2, 5, 0)  # [i, kh', kw', g, j, delta, f]
    ).reshape(OH, 98, 4, OW * 2 * NF)

    w_dev = np.zeros((NROWS * NCORES, 5, 98, 4, 1024), dtype=bf16)
    for t5 in range(5):
        p0 = 8 * t5
        npat = 8 if t5 < 4 else 7
        w_dev[:OH, t5, :, :, :npat * 128] = \
            w_flat[:, :, :, 128 * p0: 128 * (p0 + npat)]

    in_maps = []
    for c in range(NCORES):
        in_maps.append({
            "x": np.ascontiguousarray(x_dev[10 * c: 10 * c + NGROUPS]),
            "w": np.ascontiguousarray(w_dev[NROWS * c: NROWS * (c + 1)]),
        })
    return in_maps


def _postprocess(results, bias):
    """results: list of per-core dicts with 'y' [NROWS, 128, OW*NF] bf16."""
    y = np.stack([np.asarray(r["y"], dtype=np.float32) for r in results])
    y = y.reshape(NCORES * NROWS, B, OW, NF)[:OH]    # [39, 128, 39, 64]
    out = np.ascontiguousarray(y.transpose(1, 3, 0, 2)).reshape(B, NF * P)
    out = out + np.asarray(bias, dtype=np.float32).reshape(1, NF * P)
    return out.reshape(B, NF * P, 1)


def kernel(x, weight, bias):
    from concourse.bass_utils import run_bass_kernel_spmd

    if "nc" not in _CACHE:
        _CACHE["nc"] = build_program()
    nc = _CACHE["nc"]
    in_maps = _preprocess(x, weight)
    res = run_bass_kernel_spmd(nc, in_maps, core_ids=list(range(NCORES)))
    return _postprocess(res.results, bias)


# revision 12
# speedup vs baseline: 1.0384x; 1.0261x over previous
"""Trainium2 Bass kernel for nn_Conv2d_Local (locally-connected conv, untied
weights).

Problem: x [B=128, 1, 560, 560]; weight [P*NF, 1, 28, 28] with P=39*39=1521
patch locations (stride 14, kernel 28), NF=64 filters; bias [P*NF, 1].
out[b, f*P+p] = sum_{kh,kw} x[b, i*14+kh, j*14+kw] * w[f*P+p, kh, kw] + bias.

Strategy: shard the 39 patch rows across 8 cores (5 rows each, row 39 padded).
Per patch p this is a GEMM patch[b, 784] @ w_p[784, 64]. The contraction is
chunked as 4 kh-groups of 7 rows x 2 kw-blocks of 14 cols (K=98 per chunk,
aligned to the stride so x chunks are shared between horizontally adjacent
patches). Adjacent patches' chunks that share the same x tile are paired into
one matmul of N=128 (two 64-wide weight halves -> two adjacent 64-col psum
slices), so each x tile is loaded stationary exactly once per patch row.

All device data is bf16 (inputs quantized host-side; psum accumulates fp32;
output stored bf16 and upcast on host), which quarters tensor-engine time and
halves HBM traffic vs fp32. Input DMAs ride both HWDGE rings: x slabs on the
SP ring (nc.sync), weights on the ACT ring (nc.scalar); output on SWDGE
(gpsimd). Weight DMAs are one 784KB transfer per (row, psum-bank); the first
row stays at pair granularity (and the first x slab is loaded in five column
slices) so compute starts early. Outputs drain per psum-bank as soon as each
bank's accumulation closes. Host pre-permutes x and w into DMA-friendly
layouts, adds the bias, and reassembles the final output in fp32.
"""
import sys

if '/opt/trn_rl_repo' not in sys.path:
    sys.path.insert(0, '/opt/trn_rl_repo')

import numpy as np

B = 128
H = W = 560
KH = KW = 28
DH = DW = 14
NF = 64
OH = OW = 39
P = OH * OW
NCORES = 8
NROWS = 5          # patch rows per core (40 total, row 39 is padding)
NGROUPS = 12       # 7-row kh-groups per core: rows 2*ri .. 2*ri+3 per patch row
GROWS = 82         # global 7-row groups covering 574 (padded) x rows
XSLAB_BUFS = 9
WH_BUFS = 20       # weight pair tiles [98, 2, 1024] bf16, 10 consumed per row
OROW_BUFS = 2
PSUM_BUFS = 8

_CACHE = {}


def build_program(repeats: int = 1):
    import concourse.bacc as bacc
    import concourse.mybir as mybir
    from concourse.tile import TileContext

    f32 = mybir.dt.float32
    bf16 = mybir.dt.bfloat16
    nc = bacc.Bacc("TRN2", target_bir_lowering=False, debug=False,
                   num_devices=NCORES)
    # DRAM layouts carry a small per-partition gap (the trailing pad columns)
    # so every DMA splits into 2 descriptors per partition: 196 descriptors
    # spread over all 16 SDMA engines instead of 98 over 14.
    x_in = nc.dram_tensor("x", [NGROUPS, 98, 2, 21, 128], bf16,
                          kind="ExternalInput")
    w_in = nc.dram_tensor("w", [NROWS, 5, 98, 4, 1088], bf16,
                          kind="ExternalInput")
    y_out = nc.dram_tensor("y", [NROWS, 128, OW * NF], bf16, kind="ExternalOutput")

    with TileContext(nc) as tc:
        with tc.tile_pool(name="xslab", bufs=XSLAB_BUFS) as xpool, \
             tc.tile_pool(name="wh", bufs=WH_BUFS) as wpool, \
             tc.tile_pool(name="op", bufs=OROW_BUFS) as opool, \
             tc.tile_pool(name="ps", bufs=PSUM_BUFS, space="PSUM") as pspool:
            def block_slices(mrel, npat):
                """(weight col slice, psum col slice) for block mrel of a bank."""
                if mrel == 0:
                    return (0, 64), (0, 64)
                if mrel < npat:
                    return ((128 * mrel - 64, 128 * mrel + 64),
                            (64 * (mrel - 1), 64 * (mrel + 1)))
                return ((128 * npat - 64, 128 * npat),
                        (64 * (npat - 1), 64 * npat))

            for _rep in range(repeats):
                xslabs = {}
                wtiles = {}

                def load_slab(gi, sliced=False):
                    # x slabs ride the SP HWDGE ring (nc.sync); emission is
                    # demand-ordered so the per-ring FIFO matches first use.
                    # The very first slab lands in five column slices so the
                    # first matmul only gates on ~230KB.
                    if gi in xslabs or gi > 2 * (NROWS - 1) + 3:
                        return
                    t = xpool.tile([98, 2, 20, 128], bf16, tag="xslab",
                                   name=f"xs{gi}")
                    if sliced:
                        for q in (0, 1):
                            for lo, hi in ((0, 10), (10, 20)):
                                nc.sync.dma_start(
                                    out=t[:, q, lo:hi, :],
                                    in_=x_in[gi, :, q, lo:hi, :])
                    else:
                        nc.sync.dma_start(out=t, in_=x_in[gi, :, :, :20, :])
                    xslabs[gi] = t

                def load_w(ri, t5, pair, eng):
                    # t5=4 weight tiles ride the SP ring to byte-balance the
                    # two HWDGE rings (ACT: 8 tiles/row, SP: 2 tiles + slabs)
                    npat = 8 if t5 < 4 else 7
                    npc = npat * 128
                    wt = wpool.tile([98, 2, 1024], bf16, tag="wh",
                                    name=f"wh{ri}_{t5}_{pair}")
                    eng.dma_start(
                        out=wt[:, :, :npc],
                        in_=w_in[ri, t5, :, 2 * pair: 2 * pair + 2, :npc])
                    wtiles[(ri, t5, pair)] = wt

                # row-0 prologue on the SP ring, in demand order
                load_slab(0, sliced=(_rep == 0))
                load_w(0, 4, 0, nc.sync)
                load_slab(1)
                load_w(0, 4, 1, nc.sync)
                for g in (2, 3, 4, 5):
                    load_slab(g)

                for ri in range(NROWS):
                    if ri >= 1:
                        # row segment on the SP ring: this row's t5=4 weights
                        # first (needed soonest), then next-row slab prefetch
                        load_w(ri, 4, 0, nc.sync)
                        load_w(ri, 4, 1, nc.sync)
                        load_slab(2 * ri + 4)
                        load_slab(2 * ri + 5)
                    last_row = (ri == NROWS - 1)
                    if not last_row:
                        orow = opool.tile([128, OW * NF], bf16, tag="orow",
                                          name=f"orow{ri}")
                    psrow = [pspool.tile([128, 512], f32, tag="ps",
                                         name=f"ps{ri}_{t5}")
                             for t5 in range(5)]
                    for g4 in range(4):
                        pair, gg = divmod(g4, 2)
                        slab = xslabs[2 * ri + g4]
                        for t5 in range(5):
                            p0 = 8 * t5
                            npat = 8 if t5 < 4 else 7
                            if gg == 0 and t5 < 4:
                                load_w(ri, t5, pair, nc.scalar)
                            wt = wtiles[(ri, t5, pair)]
                            for mrel in range(npat + 1):
                                m = p0 + mrel
                                wsl, osl = block_slices(mrel, npat)
                                start = (g4 == 0 and mrel == 0)
                                stop = (g4 == 3 and mrel == npat)
                                nc.tensor.matmul(
                                    psrow[t5][:, osl[0]:osl[1]],
                                    slab[:, m // 20, m % 20, :],
                                    wt[:, gg, wsl[0]:wsl[1]],
                                    start=start, stop=stop)
                            if g4 == 3:
                                # drain this bank now: the cast overlaps the
                                # remaining banks' matmuls; the last row also
                                # stores per-bank so the final DMA is small
                                if last_row:
                                    ot = opool.tile([128, 512], bf16,
                                                    tag="orow",
                                                    name=f"o{ri}_{t5}")
                                    nc.vector.tensor_copy(
                                        out=ot[:, :npat * 64],
                                        in_=psrow[t5][:, :npat * 64])
                                    nc.gpsimd.dma_start(
                                        out=y_out[ri, :,
                                                  512 * t5: 512 * t5 + npat * 64],
                                        in_=ot[:, :npat * 64])
                                else:
                                    nc.vector.tensor_copy(
                                        out=orow[:, 512 * t5: 512 * t5 + npat * 64],
                                        in_=psrow[t5][:, :npat * 64])
                    if not last_row:
                        nc.gpsimd.dma_start(out=y_out[ri], in_=orow)
    nc.finalize()
    return nc


def _preprocess(x, weight):
    """Build per-core bf16 input maps from full x [B,1,560,560],
    weight [P*NF,1,28,28]."""
    import ml_dtypes
    bf16 = ml_dtypes.bfloat16

    x = np.asarray(x, dtype=np.float32).astype(bf16)
    weight = np.asarray(weight, dtype=np.float32).astype(bf16)

    # x -> pixel-major [574(pad), 560, 128], then 7-row slabs with partition
    # order (kh', kw'): [82, 98, 40, 128]; store with a pad block after each
    # 20-block half so every DMA has 2 descriptors per partition
    xt = np.zeros((GROWS * 7, W, B), dtype=bf16)
    xt[:H] = x[:, 0].transpose(1, 2, 0)
    x_flat = np.ascontiguousarray(
        xt.reshape(GROWS, 7, 40, 14, B).transpose(0, 1, 3, 2, 4)
    ).reshape(GROWS, 98, 40, 128)
    x_dev = np.zeros((GROWS, 98, 2, 21, 128), dtype=bf16)
    x_dev[:, :, 0, :20] = x_flat[:, :, 0:20]
    x_dev[:, :, 1, :20] = x_flat[:, :, 20:40]

    # weight rows are f*P + p; reshape kh=(g,kh'), kw=(delta,kw') and order as
    # [i, k=(kh',kw'), g, cols=(j, delta, f)]
    w6 = weight.reshape(NF, OH, OW, 4, 7, 2, 14)
    w_flat = np.ascontiguousarray(
        w6.transpose(1, 4, 6, 3, 2, 5, 0)  # [i, kh', kw', g, j, delta, f]
    ).reshape(OH, 98, 4, OW * 2 * NF)

    w_dev = np.zeros((NROWS * NCORES, 5, 98, 4, 1088), dtype=bf16)
    for t5 in range(5):
        p0 = 8 * t5
        npat = 8 if t5 < 4 else 7
        w_dev[:OH, t5, :, :, :npat * 128] = \
            w_flat[:, :, :, 128 * p0: 128 * (p0 + npat)]

    in_maps = []
    for c in range(NCORES):
        in_maps.append({
            "x": np.ascontiguousarray(x_dev[10 * c: 10 * c + NGROUPS]),
            "w": np.ascontiguousarray(w_dev[NROWS * c: NROWS * (c + 1)]),
        })
    return in_maps


def _postprocess(results, bias):
    """results: list of per-core dicts with 'y' [NROWS, 128, OW*NF] bf16."""
    y = np.stack([np.asarray(r["y"], dtype=np.float32) for r in results])
    y = y.reshape(NCORES * NROWS, B, OW, NF)[:OH]    # [39, 128, 39, 64]
    out = np.ascontiguousarray(y.transpose(1, 3, 0, 2)).reshape(B, NF * P)
    out = out + np.asarray(bias, dtype=np.float32).reshape(1, NF * P)
    return out.reshape(B, NF * P, 1)


def kernel(x, weight, bias):
    from concourse.bass_utils import run_bass_kernel_spmd

    if "nc" not in _CACHE:
        _CACHE["nc"] = build_program()
    nc = _CACHE["nc"]
    in_maps = _preprocess(x, weight)
    res = run_bass_kernel_spmd(nc, in_maps, core_ids=list(range(NCORES)))
    return _postprocess(res.results, bias)


# revision 14
# speedup vs baseline: 1.1243x; 1.0827x over previous
"""Trainium2 Bass kernel for nn_Conv2d_Local (locally-connected conv, untied
weights).

Problem: x [B=128, 1, 560, 560]; weight [P*NF, 1, 28, 28] with P=39*39=1521
patch locations (stride 14, kernel 28), NF=64 filters; bias [P*NF, 1].
out[b, f*P+p] = sum_{kh,kw} x[b, i*14+kh, j*14+kw] * w[f*P+p, kh, kw] + bias.

Strategy: shard the 39 patch rows across 8 cores (5 rows each, row 39 padded).
Per patch p this is a GEMM patch[b, 784] @ w_p[784, 64]. The contraction is
chunked along kh into per-14-row-period groups of {8,6} rows x 14 kw cols:
K=112 ("A") and K=84 ("B") chunks. 112 = 7x16 spreads each A DMA perfectly
across all 16 SDMA engines (the DMA splitter assigns ceil(P/16) partitions
per engine, so 98-partition tiles would ride only 14 engines). Chunks align
to the stride so x slabs are shared between vertically adjacent patch rows,
and adjacent patches' chunks that share an x column-block are paired into one
matmul of N=128 (two 64-wide weight halves -> two adjacent 64-col psum
slices), so each x block is loaded stationary exactly once per patch row.

All device data is bf16 (inputs quantized host-side; psum accumulates fp32;
output stored bf16 and upcast on host), which quarters tensor-engine time and
halves HBM traffic vs fp32. Input DMAs ride both HWDGE rings in demand order:
x slabs + t5=4 weight tiles on the SP ring (nc.sync), remaining weights on
the ACT ring (nc.scalar), byte-balanced so both rings drain together; output
on SWDGE (gpsimd). Outputs drain per psum bank; the last row stores per-bank
so the final DMA is small. Host pre-permutes x and w into these layouts,
adds the bias, and reassembles the final output in fp32.
"""
import sys

if '/opt/trn_rl_repo' not in sys.path:
    sys.path.insert(0, '/opt/trn_rl_repo')

import numpy as np

B = 128
H = W = 560
KH = KW = 28
DH = DW = 14
NF = 64
OH = OW = 39
P = OH * OW
NCORES = 8
NROWS = 5          # patch rows per core (40 total, row 39 is padding)
NPER = 6           # 14-row x periods per core (5 rows + 1 lookahead)
GPER = 41          # global 14-row periods covering 574 (padded) x rows
KA = 112           # A-chunk: 8 kh rows x 14 kw cols
KB = 84            # B-chunk: 6 kh rows x 14 kw cols
XA_BUFS = 4
XB_BUFS = 4
WA_BUFS = 10
WB_BUFS = 10
OROW_BUFS = 2
OT_BUFS = 5
PSUM_BUFS = 8

_CACHE = {}


def build_program(repeats: int = 1):
    import concourse.bacc as bacc
    import concourse.mybir as mybir
    from concourse.tile import TileContext

    f32 = mybir.dt.float32
    bf16 = mybir.dt.bfloat16
    nc = bacc.Bacc("TRN2", target_bir_lowering=False, debug=False,
                   num_devices=NCORES)
    xa_in = nc.dram_tensor("xa", [NPER, KA, 40, 128], bf16, kind="ExternalInput")
    xb_in = nc.dram_tensor("xb", [NPER, KB, 40, 128], bf16, kind="ExternalInput")
    wa_in = nc.dram_tensor("wa", [NROWS, 5, KA, 2, 1024], bf16,
                           kind="ExternalInput")
    wb_in = nc.dram_tensor("wb", [NROWS, 5, KB, 2, 1024], bf16,
                           kind="ExternalInput")
    y_out = nc.dram_tensor("y", [NROWS, 128, OW * NF], bf16, kind="ExternalOutput")

    with TileContext(nc) as tc:
        with tc.tile_pool(name="xa", bufs=XA_BUFS) as xapool, \
             tc.tile_pool(name="xb", bufs=XB_BUFS) as xbpool, \
             tc.tile_pool(name="wa", bufs=WA_BUFS) as wapool, \
             tc.tile_pool(name="wb", bufs=WB_BUFS) as wbpool, \
             tc.tile_pool(name="op", bufs=OROW_BUFS) as opool, \
             tc.tile_pool(name="ot", bufs=OT_BUFS) as otpool, \
             tc.tile_pool(name="ps", bufs=PSUM_BUFS, space="PSUM") as pspool:
            def block_slices(mrel, npat):
                """(weight col slice, psum col slice) for block mrel of a bank."""
                if mrel == 0:
                    return (0, 64), (0, 64)
                if mrel < npat:
                    return ((128 * mrel - 64, 128 * mrel + 64),
                            (64 * (mrel - 1), 64 * (mrel + 1)))
                return ((128 * npat - 64, 128 * npat),
                        (64 * (npat - 1), 64 * npat))

            for _rep in range(repeats):
                slabs = {}
                wtiles = {}

                def load_slab(kind, pi):
                    # x slabs ride the SP HWDGE ring (nc.sync); emission is
                    # demand-ordered so the per-ring FIFO matches first use
                    if (kind, pi) in slabs or pi >= NPER:
                        return
                    pool, src, k = ((xapool, xa_in, KA) if kind == 'a'
                                    else (xbpool, xb_in, KB))
                    t = pool.tile([k, 40, 128], bf16, tag=f"x{kind}",
                                  name=f"x{kind}{pi}")
                    nc.sync.dma_start(out=t, in_=src[pi])
                    slabs[(kind, pi)] = t

                def load_w(kind, ri, t5, eng):
                    # t5=4 weight tiles ride the SP ring to byte-balance the
                    # two HWDGE rings
                    npat = 8 if t5 < 4 else 7
                    npc = npat * 128
                    pool, src, k = ((wapool, wa_in, KA) if kind == 'a'
                                    else (wbpool, wb_in, KB))
                    wt = pool.tile([k, 2, 1024], bf16, tag=f"w{kind}",
                                   name=f"w{kind}{ri}_{t5}")
                    eng.dma_start(out=wt[:, :, :npc],
                                  in_=src[ri, t5, :, :, :npc])
                    wtiles[(kind, ri, t5)] = wt

                # row-0 prologue on the SP ring, in demand order
                load_slab('a', 0)
                load_w('a', 0, 4, nc.sync)
                load_slab('b', 0)
                load_w('b', 0, 4, nc.sync)
                load_slab('a', 1)
                load_slab('b', 1)
                load_slab('a', 2)
                load_slab('b', 2)

                for ri in range(NROWS):
                    if ri >= 1:
                        # row segment on the SP ring: this row's t5=4 weights
                        # first (needed soonest), then next-row slab prefetch
                        load_w('a', ri, 4, nc.sync)
                        load_w('b', ri, 4, nc.sync)
                        load_slab('a', ri + 2)
                        load_slab('b', ri + 2)
                    last_row = (ri == NROWS - 1)
                    if not last_row:
                        orow = opool.tile([128, OW * NF], bf16, tag="orow",
                                          name=f"orow{ri}")
                    psrow = [pspool.tile([128, 512], f32, tag="ps",
                                         name=f"ps{ri}_{t5}")
                             for t5 in range(5)]
                    # sub-pass order: (A, ri), (B, ri), (A, ri+1), (B, ri+1)
                    for g4 in range(4):
                        kind = 'ab'[g4 % 2]
                        per = g4 // 2
                        slab = slabs[(kind, ri + per)]
                        for t5 in range(5):
                            p0 = 8 * t5
                            npat = 8 if t5 < 4 else 7
                            if g4 == 0 and t5 < 4:
                                load_w('a', ri, t5, nc.scalar)
                                load_w('b', ri, t5, nc.scalar)
                            wt = wtiles[(kind, ri, t5)]
                            for mrel in range(npat + 1):
                                m = p0 + mrel
                                wsl, osl = block_slices(mrel, npat)
                                start = (g4 == 0 and mrel == 0)
                                stop = (g4 == 3 and mrel == npat)
                                nc.tensor.matmul(
                                    psrow[t5][:, osl[0]:osl[1]],
                                    slab[:, m, :],
                                    wt[:, per, wsl[0]:wsl[1]],
                                    start=start, stop=stop)
                            if g4 == 3:
                                # drain this bank now: the cast overlaps the
                                # remaining banks' matmuls; the last row also
                                # stores per-bank so the final DMA is small
                                if last_row:
                                    ot = otpool.tile([128, 512], bf16,
                                                     tag="ot",
                                                     name=f"o{ri}_{t5}")
                                    nc.vector.tensor_copy(
                                        out=ot[:, :npat * 64],
                                        in_=psrow[t5][:, :npat * 64])
                                    nc.gpsimd.dma_start(
                                        out=y_out[ri, :,
                                                  512 * t5: 512 * t5 + npat * 64],
                                        in_=ot[:, :npat * 64])
                                else:
                                    nc.vector.tensor_copy(
                                        out=orow[:, 512 * t5: 512 * t5 + npat * 64],
                                        in_=psrow[t5][:, :npat * 64])
                    if not last_row:
                        nc.gpsimd.dma_start(out=y_out[ri], in_=orow)
    nc.finalize()
    return nc


def _preprocess(x, weight):
    """Build per-core bf16 input maps from full x [B,1,560,560],
    weight [P*NF,1,28,28]."""
    import ml_dtypes
    bf16 = ml_dtypes.bfloat16

    x = np.asarray(x, dtype=np.float32).astype(bf16)
    weight = np.asarray(weight, dtype=np.float32).astype(bf16)

    # x -> pixel-major [574(pad), 560, 128], then per-14-row-period slabs of
    # {8,6} rows with partition order (row_in_group, kw'):
    #   xA [41, 112, 40, 128], xB [41, 84, 40, 128]
    xt = np.zeros((GPER * 14, W, B), dtype=bf16)
    xt[:H] = x[:, 0].transpose(1, 2, 0)
    x6 = xt.reshape(GPER, 14, 40, 14, B)
    xa = np.ascontiguousarray(
        x6[:, :8].transpose(0, 1, 3, 2, 4)).reshape(GPER, KA, 40, 128)
    xb = np.ascontiguousarray(
        x6[:, 8:].transpose(0, 1, 3, 2, 4)).reshape(GPER, KB, 40, 128)

    # weight rows are f*P + p; reshape kh=(period, r), kw=(delta, kw') and
    # order as [i, k=(r,kw'), period, cols=(j, delta, f)] for r in the A (8)
    # and B (6) row groups
    w7 = weight.reshape(NF, OH, OW, 2, 14, 2, 14)  # [f,i,j,per,r,delta,kw']
    wa_flat = np.ascontiguousarray(
        w7[:, :, :, :, :8].transpose(1, 4, 6, 3, 2, 5, 0)
    ).reshape(OH, KA, 2, OW * 2 * NF)
    wb_flat = np.ascontiguousarray(
        w7[:, :, :, :, 8:].transpose(1, 4, 6, 3, 2, 5, 0)
    ).reshape(OH, KB, 2, OW * 2 * NF)

    wa_dev = np.zeros((NROWS * NCORES, 5, KA, 2, 1024), dtype=bf16)
    wb_dev = np.zeros((NROWS * NCORES, 5, KB, 2, 1024), dtype=bf16)
    for t5 in range(5):
        p0 = 8 * t5
        npat = 8 if t5 < 4 else 7
        wa_dev[:OH, t5, :, :, :npat * 128] = \
            wa_flat[:, :, :, 128 * p0: 128 * (p0 + npat)]
        wb_dev[:OH, t5, :, :, :npat * 128] = \
            wb_flat[:, :, :, 128 * p0: 128 * (p0 + npat)]

    in_maps = []
    for c in range(NCORES):
        # core c covers periods 5c..5c+5; core 7's last period is 40 = GPER-1
        in_maps.append({
            "xa": np.ascontiguousarray(xa[5 * c: 5 * c + NPER]),
            "xb": np.ascontiguousarray(xb[5 * c: 5 * c + NPER]),
            "wa": np.ascontiguousarray(wa_dev[NROWS * c: NROWS * (c + 1)]),
            "wb": np.ascontiguousarray(wb_dev[NROWS * c: NROWS * (c + 1)]),
        })
    return in_maps


def _postprocess(results, bias):
    """results: list of per-core dicts with 'y' [NROWS, 128, OW*NF] bf16."""
    y = np.stack([np.asarray(r["y"], dtype=np.float32) for r in results])
    y = y.reshape(NCORES * NROWS, B, OW, NF)[:OH]    # [39, 128, 39, 64]
    out = np.ascontiguousarray(y.transpose(1, 3, 0, 2)).reshape(B, NF * P)
    out = out + np.asarray(bias, dtype=np.float32).reshape(1, NF * P)
    return out.reshape(B, NF * P, 1)


def kernel(x, weight, bias):
    from concourse.bass_utils import run_bass_kernel_spmd

    if "nc" not in _CACHE:
        _CACHE["nc"] = build_program()
    nc = _CACHE["nc"]
    in_maps = _preprocess(x, weight)
    res = run_bass_kernel_spmd(nc, in_maps, core_ids=list(range(NCORES)))
    return _postprocess(res.results, bias)
